# revision 16
# baseline (speedup 1.0000x reference)
"""Multi-head self-attention (B=4, N=2048, C=768, H=12, causal + RoPE) on 8 TRN2 cores.

Sharding: core = (batch b = core // 2, head-group g = core % 2); each core computes
6 heads of one batch end-to-end (qkv -> rope -> causal flash attention -> partial
output projection over its 384 channels). Host sums the two partial projections
per batch and adds the bias.

Device layout notes:
  - everything is kept "transposed" ([channel, token]) so that attention scores
    are computed directly as scoresT[k, q] = kT' . qT' and P@V needs no transposes.
  - RoPE: q' = q*cos + swap32(q*sin_tau) using sin[d] == sin[swap32(d)]; the cos/sin
    multiplies are fused into the PSUM->SBUF evacuation (scalar_tensor_tensor), the
    swap is a free SBUF->SBUF DMA, leaving one DVE add per block.
  - V carries an extra all-ones column per head; the PV matmul then accumulates the
    softmax denominator in psum row 64 for free.
  - matmuls run as float32r (fp32 data, full PE rate at free-dim >= 256).
  - PSUM: 2 banks qkv/proj accumulators + 4 banks score supers (double buffered)
    + 2 banks PV accumulators = 8.
"""

import os
import sys

import numpy as np

sys.path.insert(0, "/opt/trn_rl_repo")

import concourse.bass as bass
import concourse.mybir as mybir
import concourse.tile as tile
from concourse import bacc
from concourse.bass_utils import run_bass_kernel_spmd

dt = mybir.dt
F32 = dt.float32
F32R = dt.float32r
BF16 = dt.bfloat16
AF = mybir.ActivationFunctionType
ALU = mybir.AluOpType

B, N, C = 4, 2048, 768
H, HD = 12, 64
HL = 6            # heads per core
G = 2             # head groups (cores per batch)
NCORES = 8
NT = N // 128     # 16 n-tiles
QB = 512          # query block
NQB = N // QB     # 4 query blocks
CT = C // 128     # 6 contraction tiles of x channels
DL = HL * HD      # 384 local channels
VW = HL * (HD + 1)  # 390: v columns + ones column per head
SCALE = float(HD) ** -0.5

LAST_RESULTS = None  # test harness can read exec_time_ns etc. from here


def _emit(nc, tc, dram):
    xT_d, wqT_d, wkT_d, wvaT_d, cosT2_d, sinT2t_d, projwT_d, outp_d = dram

    with tc.tile_pool(name="persist", bufs=1) as pp:
        qT = [pp.tile([128, N], BF16, tag=f"qT{t}", name=f"qT{t}") for t in range(3)]
        kT = [pp.tile([128, N], BF16, tag=f"kT{t}", name=f"kT{t}") for t in range(3)]
        V = [pp.tile([128, VW], BF16, tag=f"V{t}", name=f"V{t}") for t in range(NT)]
        attnT = [pp.tile([128, N], BF16, tag=f"aT{t}", name=f"aT{t}") for t in range(3)]

        # attention working tiles live in the persistent pool so their writes
        # are never gated on the qkv pool's address release
        tri = pp.tile([128, 128], F32, tag="tri", name="tri")
        nc.gpsimd.memset(tri[:], 0.0)
        nc.gpsimd.affine_select(
            out=tri[:],
            in_=tri[:],
            compare_op=ALU.is_ge,
            fill=-1e30,
            base=0,
            pattern=[[1, 128]],
            channel_multiplier=-1,
        )

        with tc.tile_pool(name="mm_ps", bufs=1, space="PSUM") as mmp:
            # ---------------- phase 1: qkv + rope ----------------
            with tc.tile_pool(name="qkv_sb", bufs=1) as wp:
                wq = [
                    wp.tile([128, DL], BF16, tag=f"wq{t}", name=f"wq{t}")
                    for t in range(CT)
                ]
                wk = [
                    wp.tile([128, DL], BF16, tag=f"wk{t}", name=f"wk{t}")
                    for t in range(CT)
                ]
                wva = [
                    wp.tile([128, VW], BF16, tag=f"wva{t}", name=f"wva{t}")
                    for t in range(CT)
                ]
                cosT2 = wp.tile([128, N], F32, tag="cosT2", name="cosT2")
                sinT2t = wp.tile([128, N], F32, tag="sinT2t", name="sinT2t")
                for t in range(CT):
                    nc.sync.dma_start(wq[t][:], wqT_d[128 * t : 128 * (t + 1), :])
                    nc.sync.dma_start(wk[t][:], wkT_d[128 * t : 128 * (t + 1), :])
                    nc.sync.dma_start(wva[t][:], wvaT_d[128 * t : 128 * (t + 1), :])
                nc.sync.dma_start(cosT2[:], cosT2_d[:])
                nc.sync.dma_start(sinT2t[:], sinT2t_d[:])

                for ps_idx in range(3):  # pass p computes V (p==0) + q/k d-tile p
                    for nb in range(NQB):
                        nsl = slice(QB * nb, QB * (nb + 1))
                        xtb = [
                            wp.tile(
                                [128, QB], BF16, tag="xtb", bufs=12,
                                name=f"xtb{ps_idx}_{nb}_{t}",
                            )
                            for t in range(CT)
                        ]
                        for t in range(CT):
                            nc.sync.dma_start(
                                xtb[t][:], xT_d[128 * t : 128 * (t + 1), nsl]
                            )
                        if ps_idx == 0:
                            # V for the 4 n-tiles of this block
                            for sub in range(4):
                                nt = 4 * nb + sub
                                ps = mmp.tile([128, VW], F32, tag="mm", name=f"ps_v{nt}")
                                for ct in range(CT):
                                    nc.tensor.matmul(
                                        ps[:],
                                        xtb[ct][:, 128 * sub : 128 * (sub + 1)],
                                        wva[ct][:],
                                        start=(ct == 0),
                                        stop=(ct == CT - 1),
                                    )
                                nc.vector.tensor_copy(V[nt][:], ps[:])
                                ones_cols = V[nt][:].rearrange(
                                    "p (h w) -> p h w", w=HD + 1
                                )[:, :, HD : HD + 1]
                                nc.gpsimd.memset(ones_cols, 1.0)
                        # q/k d-tile ps_idx with fused rope evacuation
                        dtile = ps_idx
                        for mat, w, dest in (("q", wq, qT), ("k", wk, kT)):
                            ps = mmp.tile(
                                [128, QB], F32, tag="mm",
                                name=f"ps_{mat}{ps_idx}_{nb}",
                            )
                            for ct in range(CT):
                                nc.tensor.matmul(
                                    ps[:],
                                    w[ct][:, 128 * dtile : 128 * (dtile + 1)],
                                    xtb[ct][:],
                                    start=(ct == 0),
                                    stop=(ct == CT - 1),
                                )
                            dst = dest[dtile][:, nsl]
                            # dst = psum * cos ; wsin = psum * sin_tau
                            nc.vector.scalar_tensor_tensor(
                                out=dst,
                                in0=ps[:],
                                scalar=1.0,
                                in1=cosT2[:, nsl],
                                op0=ALU.mult,
                                op1=ALU.mult,
                            )
                            wsin = wp.tile(
                                [128, QB], BF16, tag="wsin", bufs=2,
                                name=f"ws_{mat}{ps_idx}_{nb}",
                            )
                            nc.vector.scalar_tensor_tensor(
                                out=wsin[:],
                                in0=ps[:],
                                scalar=1.0,
                                in1=sinT2t[:, nsl],
                                op0=ALU.mult,
                                op1=ALU.mult,
                            )
                            wrot = wp.tile(
                                [128, QB], BF16, tag="wrot", bufs=2,
                                name=f"wr_{mat}{ps_idx}_{nb}",
                            )
                            for blk in range(4):
                                lo = 32 * blk
                                swp = 32 * (blk + 1) if blk % 2 == 0 else 32 * (blk - 1)
                                nc.sync.dma_start(
                                    wrot[lo : lo + 32, :], wsin[swp : swp + 32, :]
                                )
                            nc.vector.tensor_add(dst, dst, wrot[:])

            # ---------------- phase 2: causal attention ----------------
            with (
                tc.tile_pool(name="score_ps", bufs=2, space="PSUM") as sp,
                tc.tile_pool(name="out_ps", bufs=3, space="PSUM") as op,
            ):
                ap = pp
                for pt in range(3):  # head pair (local heads 2pt, 2pt+1)
                    hA, hB = 2 * pt, 2 * pt + 1
                    for qb in range(NQB):
                        qsl = slice(QB * qb, QB * (qb + 1))
                        nkt = 4 * qb + 4  # causal: k-tiles 0 .. 4qb+3
                        psA = op.tile([65, QB], F32, tag="outps", name=f"psA{pt}_{qb}")
                        psB = op.tile([65, QB], F32, tag="outps", name=f"psB{pt}_{qb}")
                        for kt in range(nkt):
                            ksl = slice(128 * kt, 128 * (kt + 1))
                            S = sp.tile(
                                [128, 2 * QB], F32, tag="sc", name=f"S{pt}_{qb}_{kt}"
                            )
                            # scoresT[k, q]: head A in cols 0:512, head B in 512:1024
                            for hh in range(2):
                                prow = slice(64 * hh, 64 * hh + 64)
                                nc.tensor.matmul(
                                    S[:, QB * hh : QB * (hh + 1)],
                                    kT[pt][prow, ksl],
                                    qT[pt][prow, qsl],
                                    start=True,
                                    stop=True,
                                )
                            a = 128 * kt - QB * qb
                            if a >= 0:  # diagonal tile: causal triangle (pre-exp)
                                for hh in range(2):
                                    rb = QB * hh
                                    nc.vector.tensor_add(
                                        S[:, rb + a : rb + a + 128],
                                        S[:, rb + a : rb + a + 128],
                                        tri[:],
                                    )
                            P = ap.tile(
                                [128, 2 * QB], BF16, tag="probs", bufs=4,
                                name=f"P{pt}_{qb}_{kt}",
                            )
                            if a > 0:
                                # masked prefix: zero first (off the exp->PV path),
                                # then exp only the valid suffix of each head region
                                for hh in range(2):
                                    rb = QB * hh
                                    nc.gpsimd.memset(P[:, rb : rb + a], 0.0)
                                for hh in range(2):
                                    rb = QB * hh
                                    nc.scalar.activation(
                                        P[:, rb + a : rb + QB],
                                        S[:, rb + a : rb + QB],
                                        AF.Exp,
                                        scale=SCALE,
                                    )
                            else:
                                nc.scalar.activation(P[:], S[:], AF.Exp, scale=SCALE)
                            first, last = kt == 0, kt == nkt - 1
                            for hh, ps_out, h in ((0, psA, hA), (1, psB, hB)):
                                nc.tensor.matmul(
                                    ps_out[:],
                                    V[kt][:, 65 * h : 65 * h + 65],
                                    P[:, QB * hh : QB * (hh + 1)],
                                    start=first,
                                    stop=last,
                                    skip_group_check=True,
                                )
                        # normalize: attnT rows = out / den (dens of A,B batched)
                        dcp = ap.tile(
                            [128, 2 * QB], F32, tag="dcp", bufs=1, name=f"dcp{pt}_{qb}"
                        )
                        nc.vector.tensor_copy(dcp[64:65, 0:QB], psA[64:65, :])
                        nc.vector.tensor_copy(dcp[64:65, QB : 2 * QB], psB[64:65, :])
                        nc.sync.dma_start(dcp[0:1, :], dcp[64:65, :])
                        dr = ap.tile(
                            [128, 2 * QB], F32, tag="dr", bufs=1, name=f"dr{pt}_{qb}"
                        )
                        nc.vector.reciprocal_approx_fast(dr[0:1, :], dcp[0:1, :])
                        r = ap.tile(
                            [64, 2 * QB], F32, tag="r", bufs=1, name=f"r{pt}_{qb}"
                        )
                        nc.gpsimd.partition_broadcast(r[0:64, :], dr[0:1, :], channels=64)
                        nc.vector.scalar_tensor_tensor(
                            out=attnT[pt][0:64, qsl],
                            in0=psA[0:64, :],
                            scalar=1.0,
                            in1=r[0:64, 0:QB],
                            op0=ALU.mult,
                            op1=ALU.mult,
                        )
                        tb = ap.tile(
                            [64, QB], BF16, tag="tmpB", bufs=1, name=f"tB{pt}_{qb}"
                        )
                        nc.vector.scalar_tensor_tensor(
                            out=tb[0:64, :],
                            in0=psB[0:64, :],
                            scalar=1.0,
                            in1=r[0:64, QB : 2 * QB],
                            op0=ALU.mult,
                            op1=ALU.mult,
                        )
                        nc.sync.dma_start(attnT[pt][64:128, qsl], tb[0:64, :])

            # ---------------- phase 3: output projection (partial) ----------------
            with tc.tile_pool(name="proj_sb", bufs=1) as jp:
                pw = [
                    jp.tile([128, C], BF16, tag=f"pw{t}", name=f"pw{t}")
                    for t in range(3)
                ]
                for t in range(3):
                    nc.sync.dma_start(pw[t][:], projwT_d[128 * t : 128 * (t + 1), :])
                for nt in range(NT):
                    nsl = slice(128 * nt, 128 * (nt + 1))
                    osb = jp.tile([128, C], F32, tag="osb", bufs=2, name=f"osb{nt}")
                    for half in range(2):
                        ps = mmp.tile([128, 384], F32, tag="mm", name=f"pj{half}_{nt}")
                        for ct in range(3):
                            nc.tensor.matmul(
                                ps[:],
                                attnT[ct][:, nsl],
                                pw[ct][:, 384 * half : 384 * (half + 1)],
                                start=(ct == 0),
                                stop=(ct == 2),
                                skip_group_check=True,
                            )
                        nc.vector.tensor_copy(osb[:, 384 * half : 384 * (half + 1)], ps[:])
                    nc.sync.dma_start(outp_d[nsl, :], osb[:])


def _build_program():
    nc = bacc.Bacc(
        "TRN2",
        target_bir_lowering=False,
        debug=False,
        num_devices=NCORES,
    )

    dram = (
        nc.dram_tensor("xT", [C, N], BF16, kind="ExternalInput"),
        nc.dram_tensor("wqT", [C, DL], BF16, kind="ExternalInput"),
        nc.dram_tensor("wkT", [C, DL], BF16, kind="ExternalInput"),
        nc.dram_tensor("wvaT", [C, VW], BF16, kind="ExternalInput"),
        nc.dram_tensor("cosT2", [128, N], F32, kind="ExternalInput"),
        nc.dram_tensor("sinT2t", [128, N], F32, kind="ExternalInput"),
        nc.dram_tensor("projwT", [DL, C], BF16, kind="ExternalInput"),
        nc.dram_tensor("outp", [N, C], F32, kind="ExternalOutput"),
    )

    with tile.TileContext(nc) as tc:
        _emit(nc, tc, dram)

    nc.compile()
    return nc


def _rope_tables():
    # mirror reference.rope_tables in float32 (keep the f32 product!)
    inv_freq = 1.0 / np.power(
        np.float32(10000.0), np.arange(0, HD, 2, dtype=np.float32) / np.float32(HD)
    )
    t = np.arange(N, dtype=np.float32)
    freqs = (t[:, None] * inv_freq[None, :].astype(np.float32)).astype(np.float32)
    emb = np.concatenate([freqs, freqs], axis=-1)  # [N, 64]
    return np.cos(emb).astype(np.float32), np.sin(emb).astype(np.float32)


def _make_in_maps(x, qkv_w, proj_w):
    import ml_dtypes

    bf16 = ml_dtypes.bfloat16
    cos, sin = _rope_tables()  # [N, 64]
    # cosT2[p, n] = cos[n, p % 64]
    dd = np.arange(128) % HD
    cosT2 = np.ascontiguousarray(cos.T[dd, :])  # [128, N]
    # sin_tau sign such that q' = q*cos + swap32(q*sin_tau):
    # tau(d) = +1 for d%64 < 32, -1 otherwise
    sgn = np.where((dd % HD) < (HD // 2), np.float32(1.0), np.float32(-1.0))
    sinT2t = np.ascontiguousarray(sin.T[dd, :] * sgn[:, None]).astype(np.float32)

    in_maps = []
    for core in range(NCORES):
        b, g = core // G, core % G
        heads = [g * HL + j for j in range(HL)]
        cols = np.concatenate([np.arange(HD * h, HD * h + HD) for h in heads])
        xT = np.ascontiguousarray(x[b].T).astype(bf16)
        wqT = np.ascontiguousarray(qkv_w[cols, :].T).astype(bf16)
        wkT = np.ascontiguousarray(qkv_w[C + cols, :].T).astype(bf16)
        wv = qkv_w[2 * C + cols, :]  # [384, 768]
        wvaT = np.zeros((C, VW), dtype=np.float32)  # cast below
        for j in range(HL):
            wvaT[:, 65 * j : 65 * j + HD] = wv[HD * j : HD * j + HD, :].T
        projwT = np.ascontiguousarray(proj_w[:, cols].T).astype(bf16)
        in_maps.append(
            {
                "xT": xT,
                "wqT": wqT,
                "wkT": wkT,
                "wvaT": wvaT.astype(bf16),
                "cosT2": cosT2,
                "sinT2t": sinT2t,
                "projwT": projwT,
            }
        )
    return in_maps


def _install_ntff_hook():
    """Wire the axon NTFF profiling hook if the image's antenv lacks it."""
    import types

    try:
        from antenv.axon_hooks import get_axon_ntff_profile_hook  # noqa: F401

        return True
    except ImportError:
        pass
    try:
        import antenv
        from trn_agent_boot.trn_boot import _ntff_profile_via_ctypes

        hook = _ntff_profile_via_ctypes("/opt/axon/libaxon_pjrt.so")
        mod = types.ModuleType("antenv.axon_hooks")
        holder = {"hook": hook}
        mod.set_axon_ntff_profile_hook = lambda h: holder.__setitem__("hook", h)
        mod.get_axon_ntff_profile_hook = lambda: holder["hook"]
        sys.modules["antenv.axon_hooks"] = mod
        antenv.axon_hooks = mod
        return hook is not None
    except Exception as e:  # pragma: no cover
        print(f"ntff hook install failed: {e}")
        return False


_PROGRAM = None


def kernel(x, qkv_w, proj_w, proj_b):
    global _PROGRAM, LAST_RESULTS
    x = np.asarray(x, dtype=np.float32)
    qkv_w = np.asarray(qkv_w, dtype=np.float32)
    proj_w = np.asarray(proj_w, dtype=np.float32)
    proj_b = np.asarray(proj_b, dtype=np.float32)

    if _PROGRAM is None:
        _PROGRAM = _build_program()
    nc = _PROGRAM

    in_maps = _make_in_maps(x, qkv_w, proj_w)
    trace = bool(int(os.environ.get("KERNEL_TRACE", "0")))
    if trace:
        trace = _install_ntff_hook()
    res = run_bass_kernel_spmd(nc, in_maps, list(range(NCORES)), trace=trace)
    LAST_RESULTS = res

    out = np.empty((B, N, C), dtype=np.float32)
    for b in range(B):
        out[b] = res.results[G * b]["outp"] + res.results[G * b + 1]["outp"]
    out += proj_b[None, None, :]
    return out


if __name__ == "__main__":
    x = np.random.randn(B, N, C).astype(np.float32)
    qkv_w = np.random.randn(3 * C, C).astype(np.float32)
    proj_w = np.random.randn(C, C).astype(np.float32)
    maps = _make_in_maps(x, qkv_w, proj_w)
    for k, v in maps[0].items():
        print(k, v.shape, v.dtype)


# revision 17
# speedup vs baseline: 1.3091x; 1.3091x over previous
"""Multi-head self-attention (B=4, N=2048, C=768, H=12, causal + RoPE) on 8 TRN2 cores.

Sharding: core = (batch b = core // 2, head-group g = core % 2); each core computes
6 heads of one batch end-to-end (qkv -> rope -> causal flash attention -> partial
output projection over its 384 channels). Host sums the two partial projections
per batch and adds the bias.

Device layout notes:
  - everything is kept "transposed" ([channel, token]) so that attention scores
    are computed directly as scoresT[k, q] = kT' . qT' and P@V needs no transposes.
  - RoPE: q' = q*cos + swap32(q*sin_tau) using sin[d] == sin[swap32(d)]; the cos/sin
    multiplies are fused into the PSUM->SBUF evacuation (scalar_tensor_tensor), the
    swap is a free SBUF->SBUF DMA, leaving one DVE add per block.
  - V carries an extra all-ones column per head; the PV matmul then accumulates the
    softmax denominator in psum row 64 for free.
  - matmuls run as float32r (fp32 data, full PE rate at free-dim >= 256).
  - PSUM: 2 banks qkv/proj accumulators + 4 banks score supers (double buffered)
    + 2 banks PV accumulators = 8.
"""

import os
import sys

import numpy as np

sys.path.insert(0, "/opt/trn_rl_repo")

import concourse.bass as bass
import concourse.mybir as mybir
import concourse.tile as tile
from concourse import bacc
from concourse.bass_utils import run_bass_kernel_spmd

dt = mybir.dt
F32 = dt.float32
F32R = dt.float32r
BF16 = dt.bfloat16
AF = mybir.ActivationFunctionType
ALU = mybir.AluOpType

B, N, C = 4, 2048, 768
H, HD = 12, 64
HL = 6            # heads per core
G = 2             # head groups (cores per batch)
NCORES = 8
NT = N // 128     # 16 n-tiles
QB = 512          # query block
NQB = N // QB     # 4 query blocks
CT = C // 128     # 6 contraction tiles of x channels
DL = HL * HD      # 384 local channels
VW = HL * (HD + 1)  # 390: v columns + ones column per head
SCALE = float(HD) ** -0.5

LAST_RESULTS = None  # test harness can read exec_time_ns etc. from here


def _emit(nc, tc, dram):
    xT_d, wqT_d, wkT_d, wvaT_d, cosT2_d, sinT2t_d, projwT_d, outp_d = dram

    with tc.tile_pool(name="persist", bufs=1) as pp:
        qT = [pp.tile([128, N], BF16, tag=f"qT{t}", name=f"qT{t}") for t in range(3)]
        kT = [pp.tile([128, N], BF16, tag=f"kT{t}", name=f"kT{t}") for t in range(3)]
        V = [pp.tile([128, VW], BF16, tag=f"V{t}", name=f"V{t}") for t in range(NT)]
        attnT = [pp.tile([128, N], BF16, tag=f"aT{t}", name=f"aT{t}") for t in range(3)]

        # attention working tiles live in the persistent pool so their writes
        # are never gated on the qkv pool's address release
        tri = pp.tile([128, 128], F32, tag="tri", name="tri")
        nc.gpsimd.memset(tri[:], 0.0)
        nc.gpsimd.affine_select(
            out=tri[:],
            in_=tri[:],
            compare_op=ALU.is_ge,
            fill=-1e30,
            base=0,
            pattern=[[1, 128]],
            channel_multiplier=-1,
        )

        with tc.tile_pool(name="mm_ps", bufs=1, space="PSUM") as mmp:
            # ---------------- phase 1: qkv + rope ----------------
            with tc.tile_pool(name="qkv_sb", bufs=1) as wp:
                wq = [
                    wp.tile([128, DL], BF16, tag=f"wq{t}", name=f"wq{t}")
                    for t in range(CT)
                ]
                wk = [
                    wp.tile([128, DL], BF16, tag=f"wk{t}", name=f"wk{t}")
                    for t in range(CT)
                ]
                wva = [
                    wp.tile([128, VW], BF16, tag=f"wva{t}", name=f"wva{t}")
                    for t in range(CT)
                ]
                cosT2 = wp.tile([128, N], F32, tag="cosT2", name="cosT2")
                sinT2t = wp.tile([128, N], F32, tag="sinT2t", name="sinT2t")
                for t in range(CT):
                    nc.sync.dma_start(wq[t][:], wqT_d[128 * t : 128 * (t + 1), :])
                    nc.sync.dma_start(wk[t][:], wkT_d[128 * t : 128 * (t + 1), :])
                    nc.sync.dma_start(wva[t][:], wvaT_d[128 * t : 128 * (t + 1), :])
                nc.sync.dma_start(cosT2[:], cosT2_d[:])
                nc.sync.dma_start(sinT2t[:], sinT2t_d[:])

                for nb in range(NQB):
                    nsl = slice(QB * nb, QB * (nb + 1))
                    xtb = [
                        wp.tile([128, QB], BF16, tag="xtb", bufs=12, name=f"xtb{nb}_{t}")
                        for t in range(CT)
                    ]
                    for t in range(CT):
                        nc.sync.dma_start(
                            xtb[t][:], xT_d[128 * t : 128 * (t + 1), nsl]
                        )
                    # q/k with fused rope evacuation; d-tile-major so pair 0's
                    # q and k arrive first and attention can start immediately
                    for dtile in range(3):
                        for mat, w, dest in (("q", wq, qT), ("k", wk, kT)):
                            ps = mmp.tile(
                                [128, QB], F32, tag="mm", name=f"ps_{mat}{nb}_{dtile}"
                            )
                            for ct in range(CT):
                                nc.tensor.matmul(
                                    ps[:],
                                    w[ct][:, 128 * dtile : 128 * (dtile + 1)],
                                    xtb[ct][:],
                                    start=(ct == 0),
                                    stop=(ct == CT - 1),
                                )
                            dst = dest[dtile][:, nsl]
                            nc.vector.scalar_tensor_tensor(
                                out=dst,
                                in0=ps[:],
                                scalar=1.0,
                                in1=cosT2[:, nsl],
                                op0=ALU.mult,
                                op1=ALU.mult,
                            )
                            wsin = wp.tile(
                                [128, QB], BF16, tag="wsin", bufs=2,
                                name=f"ws_{mat}{nb}_{dtile}",
                            )
                            nc.vector.scalar_tensor_tensor(
                                out=wsin[:],
                                in0=ps[:],
                                scalar=1.0,
                                in1=sinT2t[:, nsl],
                                op0=ALU.mult,
                                op1=ALU.mult,
                            )
                            wrot = wp.tile(
                                [128, QB], BF16, tag="wrot", bufs=2,
                                name=f"wr_{mat}{nb}_{dtile}",
                            )
                            for blk in range(4):
                                lo = 32 * blk
                                swp = 32 * (blk + 1) if blk % 2 == 0 else 32 * (blk - 1)
                                nc.sync.dma_start(
                                    wrot[lo : lo + 32, :], wsin[swp : swp + 32, :]
                                )
                            nc.vector.tensor_add(dst, dst, wrot[:])
                    # V for the 4 n-tiles of this block
                    for sub in range(4):
                        nt = 4 * nb + sub
                        ps = mmp.tile([128, VW], F32, tag="mm", name=f"ps_v{nt}")
                        for ct in range(CT):
                            nc.tensor.matmul(
                                ps[:],
                                xtb[ct][:, 128 * sub : 128 * (sub + 1)],
                                wva[ct][:],
                                start=(ct == 0),
                                stop=(ct == CT - 1),
                            )
                        nc.vector.tensor_copy(V[nt][:], ps[:])
                        ones_cols = V[nt][:].rearrange("p (h w) -> p h w", w=HD + 1)[
                            :, :, HD : HD + 1
                        ]
                        nc.gpsimd.memset(ones_cols, 1.0)

            # ---------------- phase 2: causal attention ----------------
            with (
                tc.tile_pool(name="score_ps", bufs=2, space="PSUM") as sp,
                tc.tile_pool(name="out_ps", bufs=3, space="PSUM") as op,
            ):
                ap = pp
                for qb in range(NQB):  # consume query blocks in production order
                    for pt in range(3):  # head pair (local heads 2pt, 2pt+1)
                        hA, hB = 2 * pt, 2 * pt + 1
                        qsl = slice(QB * qb, QB * (qb + 1))
                        nkt = 4 * qb + 4  # causal: k-tiles 0 .. 4qb+3
                        psA = op.tile([65, QB], F32, tag="outps", name=f"psA{pt}_{qb}")
                        psB = op.tile([65, QB], F32, tag="outps", name=f"psB{pt}_{qb}")
                        for kt in range(nkt):
                            ksl = slice(128 * kt, 128 * (kt + 1))
                            S = sp.tile(
                                [128, 2 * QB], F32, tag="sc", name=f"S{pt}_{qb}_{kt}"
                            )
                            # scoresT[k, q]: head A in cols 0:512, head B in 512:1024
                            for hh in range(2):
                                prow = slice(64 * hh, 64 * hh + 64)
                                nc.tensor.matmul(
                                    S[:, QB * hh : QB * (hh + 1)],
                                    kT[pt][prow, ksl],
                                    qT[pt][prow, qsl],
                                    start=True,
                                    stop=True,
                                )
                            a = 128 * kt - QB * qb
                            if a >= 0:  # diagonal tile: causal triangle (pre-exp)
                                for hh in range(2):
                                    rb = QB * hh
                                    nc.vector.tensor_add(
                                        S[:, rb + a : rb + a + 128],
                                        S[:, rb + a : rb + a + 128],
                                        tri[:],
                                    )
                            P = ap.tile(
                                [128, 2 * QB], BF16, tag="probs", bufs=4,
                                name=f"P{pt}_{qb}_{kt}",
                            )
                            if a > 0:
                                # masked prefix: zero first (off the exp->PV path),
                                # then exp only the valid suffix of each head region
                                for hh in range(2):
                                    rb = QB * hh
                                    nc.gpsimd.memset(P[:, rb : rb + a], 0.0)
                                for hh in range(2):
                                    rb = QB * hh
                                    nc.scalar.activation(
                                        P[:, rb + a : rb + QB],
                                        S[:, rb + a : rb + QB],
                                        AF.Exp,
                                        scale=SCALE,
                                    )
                            else:
                                nc.scalar.activation(P[:], S[:], AF.Exp, scale=SCALE)
                            first, last = kt == 0, kt == nkt - 1
                            for hh, ps_out, h in ((0, psA, hA), (1, psB, hB)):
                                nc.tensor.matmul(
                                    ps_out[:],
                                    V[kt][:, 65 * h : 65 * h + 65],
                                    P[:, QB * hh : QB * (hh + 1)],
                                    start=first,
                                    stop=last,
                                    skip_group_check=True,
                                )
                        # normalize: attnT rows = out / den (dens of A,B batched)
                        dcp = ap.tile(
                            [128, 2 * QB], F32, tag="dcp", bufs=1, name=f"dcp{pt}_{qb}"
                        )
                        nc.vector.tensor_copy(dcp[64:65, 0:QB], psA[64:65, :])
                        nc.vector.tensor_copy(dcp[64:65, QB : 2 * QB], psB[64:65, :])
                        nc.sync.dma_start(dcp[0:1, :], dcp[64:65, :])
                        dr = ap.tile(
                            [128, 2 * QB], F32, tag="dr", bufs=1, name=f"dr{pt}_{qb}"
                        )
                        nc.vector.reciprocal_approx_fast(dr[0:1, :], dcp[0:1, :])
                        r = ap.tile(
                            [64, 2 * QB], F32, tag="r", bufs=1, name=f"r{pt}_{qb}"
                        )
                        nc.gpsimd.partition_broadcast(r[0:64, :], dr[0:1, :], channels=64)
                        nc.vector.scalar_tensor_tensor(
                            out=attnT[pt][0:64, qsl],
                            in0=psA[0:64, :],
                            scalar=1.0,
                            in1=r[0:64, 0:QB],
                            op0=ALU.mult,
                            op1=ALU.mult,
                        )
                        tb = ap.tile(
                            [64, QB], BF16, tag="tmpB", bufs=1, name=f"tB{pt}_{qb}"
                        )
                        nc.vector.scalar_tensor_tensor(
                            out=tb[0:64, :],
                            in0=psB[0:64, :],
                            scalar=1.0,
                            in1=r[0:64, QB : 2 * QB],
                            op0=ALU.mult,
                            op1=ALU.mult,
                        )
                        nc.sync.dma_start(attnT[pt][64:128, qsl], tb[0:64, :])

            # ---------------- phase 3: output projection (partial) ----------------
            with tc.tile_pool(name="proj_sb", bufs=1) as jp:
                pw = [
                    jp.tile([128, C], BF16, tag=f"pw{t}", name=f"pw{t}")
                    for t in range(3)
                ]
                for t in range(3):
                    nc.sync.dma_start(pw[t][:], projwT_d[128 * t : 128 * (t + 1), :])
                for nt in range(NT):
                    nsl = slice(128 * nt, 128 * (nt + 1))
                    osb = jp.tile([128, C], F32, tag="osb", bufs=2, name=f"osb{nt}")
                    for half in range(2):
                        ps = mmp.tile([128, 384], F32, tag="mm", name=f"pj{half}_{nt}")
                        for ct in range(3):
                            nc.tensor.matmul(
                                ps[:],
                                attnT[ct][:, nsl],
                                pw[ct][:, 384 * half : 384 * (half + 1)],
                                start=(ct == 0),
                                stop=(ct == 2),
                                skip_group_check=True,
                            )
                        nc.vector.tensor_copy(osb[:, 384 * half : 384 * (half + 1)], ps[:])
                    nc.sync.dma_start(outp_d[nsl, :], osb[:])


def _build_program():
    nc = bacc.Bacc(
        "TRN2",
        target_bir_lowering=False,
        debug=False,
        num_devices=NCORES,
    )

    dram = (
        nc.dram_tensor("xT", [C, N], BF16, kind="ExternalInput"),
        nc.dram_tensor("wqT", [C, DL], BF16, kind="ExternalInput"),
        nc.dram_tensor("wkT", [C, DL], BF16, kind="ExternalInput"),
        nc.dram_tensor("wvaT", [C, VW], BF16, kind="ExternalInput"),
        nc.dram_tensor("cosT2", [128, N], F32, kind="ExternalInput"),
        nc.dram_tensor("sinT2t", [128, N], F32, kind="ExternalInput"),
        nc.dram_tensor("projwT", [DL, C], BF16, kind="ExternalInput"),
        nc.dram_tensor("outp", [N, C], F32, kind="ExternalOutput"),
    )

    with tile.TileContext(nc) as tc:
        _emit(nc, tc, dram)

    nc.compile()
    return nc


def _rope_tables():
    # mirror reference.rope_tables in float32 (keep the f32 product!)
    inv_freq = 1.0 / np.power(
        np.float32(10000.0), np.arange(0, HD, 2, dtype=np.float32) / np.float32(HD)
    )
    t = np.arange(N, dtype=np.float32)
    freqs = (t[:, None] * inv_freq[None, :].astype(np.float32)).astype(np.float32)
    emb = np.concatenate([freqs, freqs], axis=-1)  # [N, 64]
    return np.cos(emb).astype(np.float32), np.sin(emb).astype(np.float32)


def _make_in_maps(x, qkv_w, proj_w):
    import ml_dtypes

    bf16 = ml_dtypes.bfloat16
    cos, sin = _rope_tables()  # [N, 64]
    # cosT2[p, n] = cos[n, p % 64]
    dd = np.arange(128) % HD
    cosT2 = np.ascontiguousarray(cos.T[dd, :])  # [128, N]
    # sin_tau sign such that q' = q*cos + swap32(q*sin_tau):
    # tau(d) = +1 for d%64 < 32, -1 otherwise
    sgn = np.where((dd % HD) < (HD // 2), np.float32(1.0), np.float32(-1.0))
    sinT2t = np.ascontiguousarray(sin.T[dd, :] * sgn[:, None]).astype(np.float32)

    in_maps = []
    for core in range(NCORES):
        b, g = core // G, core % G
        heads = [g * HL + j for j in range(HL)]
        cols = np.concatenate([np.arange(HD * h, HD * h + HD) for h in heads])
        xT = np.ascontiguousarray(x[b].T).astype(bf16)
        wqT = np.ascontiguousarray(qkv_w[cols, :].T).astype(bf16)
        wkT = np.ascontiguousarray(qkv_w[C + cols, :].T).astype(bf16)
        wv = qkv_w[2 * C + cols, :]  # [384, 768]
        wvaT = np.zeros((C, VW), dtype=np.float32)  # cast below
        for j in range(HL):
            wvaT[:, 65 * j : 65 * j + HD] = wv[HD * j : HD * j + HD, :].T
        projwT = np.ascontiguousarray(proj_w[:, cols].T).astype(bf16)
        in_maps.append(
            {
                "xT": xT,
                "wqT": wqT,
                "wkT": wkT,
                "wvaT": wvaT.astype(bf16),
                "cosT2": cosT2,
                "sinT2t": sinT2t,
                "projwT": projwT,
            }
        )
    return in_maps


def _install_ntff_hook():
    """Wire the axon NTFF profiling hook if the image's antenv lacks it."""
    import types

    try:
        from antenv.axon_hooks import get_axon_ntff_profile_hook  # noqa: F401

        return True
    except ImportError:
        pass
    try:
        import antenv
        from trn_agent_boot.trn_boot import _ntff_profile_via_ctypes

        hook = _ntff_profile_via_ctypes("/opt/axon/libaxon_pjrt.so")
        mod = types.ModuleType("antenv.axon_hooks")
        holder = {"hook": hook}
        mod.set_axon_ntff_profile_hook = lambda h: holder.__setitem__("hook", h)
        mod.get_axon_ntff_profile_hook = lambda: holder["hook"]
        sys.modules["antenv.axon_hooks"] = mod
        antenv.axon_hooks = mod
        return hook is not None
    except Exception as e:  # pragma: no cover
        print(f"ntff hook install failed: {e}")
        return False


_PROGRAM = None


def kernel(x, qkv_w, proj_w, proj_b):
    global _PROGRAM, LAST_RESULTS
    x = np.asarray(x, dtype=np.float32)
    qkv_w = np.asarray(qkv_w, dtype=np.float32)
    proj_w = np.asarray(proj_w, dtype=np.float32)
    proj_b = np.asarray(proj_b, dtype=np.float32)

    if _PROGRAM is None:
        _PROGRAM = _build_program()
    nc = _PROGRAM

    in_maps = _make_in_maps(x, qkv_w, proj_w)
    trace = bool(int(os.environ.get("KERNEL_TRACE", "0")))
    if trace:
        trace = _install_ntff_hook()
    res = run_bass_kernel_spmd(nc, in_maps, list(range(NCORES)), trace=trace)
    LAST_RESULTS = res

    out = np.empty((B, N, C), dtype=np.float32)
    for b in range(B):
        out[b] = res.results[G * b]["outp"] + res.results[G * b + 1]["outp"]
    out += proj_b[None, None, :]
    return out


if __name__ == "__main__":
    x = np.random.randn(B, N, C).astype(np.float32)
    qkv_w = np.random.randn(3 * C, C).astype(np.float32)
    proj_w = np.random.randn(C, C).astype(np.float32)
    maps = _make_in_maps(x, qkv_w, proj_w)
    for k, v in maps[0].items():
        print(k, v.shape, v.dtype)
